# revision 1
# baseline (speedup 1.0000x reference)
"""DecisionTransformer Trainium2 kernel: 8 NeuronCores, SPMD.

Sharding: core c handles batch c//2 and sequence-half c%2 (768 of 1536
tokens). Within a pair of cores the full key/value sequence is rebuilt
each layer via a pair AllGather. Residual stream is kept feature-major
(X^T) in SBUF; all matmuls run in float32r (full PE rate, ~1e-4 rel err).
"""
import numpy as np
from contextlib import ExitStack

import concourse.bass as bass
import concourse.tile as tile
from concourse import bacc, mybir
from concourse.bass_utils import run_bass_kernel_spmd

B, L, D, H, FF, NL, SS, A = 4, 512, 1024, 16, 4096, 4, 128, 16
DH = 64
S = 3 * L            # 1536 tokens
T = S // 2           # 768 own tokens per core
NC = 8
KT = S // 128        # 12 key tiles
DT = D // 128        # 8 feature tiles
FC = FF // 128       # 32 ffn tiles
F32 = mybir.dt.float32
F32R = mybir.dt.float32r
U32 = mybir.dt.uint32
AF = mybir.ActivationFunctionType
ALU = mybir.AluOpType
GROUPS = [[0, 1], [2, 3], [4, 5], [6, 7]]

_CACHE = {}
_NO_CC = False
_REPS = 1
_ATT_BF16 = True
BF16 = mybir.dt.bfloat16


def _nchunks(n, c=512):
    out = []
    o = 0
    while o < n:
        w = min(c, n - o)
        out.append((o, w))
        o += w
    return out


def _emit(nc, tc, ctx, t):
    pers = ctx.enter_context(tc.tile_pool(name="pers", bufs=1))

    # ---- persistent small tiles ----
    ones_r = pers.tile([128, 1], F32R, tag="ones")
    nc.vector.memset(ones_r[:].bitcast(F32), 1.0)
    eps_t = pers.tile([1, 1], F32, tag="eps")
    nc.vector.memset(eps_t[:], 1e-5)
    maskv = pers.tile([128, KT], F32, tag="maskv")
    nc.sync.dma_start(maskv[:], t["p_maskv"][:])
    offs_t = pers.tile([1, DT], U32, tag="offs")
    nc.sync.dma_start(offs_t[:], t["p_offs"][:])

    # dynamic offsets for reading the peer half of ag_out (per-core data)
    POOL = mybir.EngineType.Pool
    dynvals = []
    for dt in range(DT):
        reg = nc.alloc_registers(engines=[POOL])
        nc.gpsimd.reg_load(reg, offs_t[0:1, dt:dt + 1])
        dynvals.append(nc.snap(reg, donate=True, min_val=0, max_val=2 * D * T))

    # ---- residual stream X^T, 8 tiles [128, S] ----
    xt = [pers.tile([128, S], F32R, tag=f"xt{i}", name=f"xt{i}")
          for i in range(DT)]

    # ---- embedding ----
    st_t = pers.tile([SS, S], F32R, tag="st")
    nc.sync.dma_start(st_t[:], t["p_state"][:].bitcast(F32R))
    ras_t = pers.tile([3, S], F32R, tag="ras")
    nc.sync.dma_start(ras_t[:], t["p_ras"][:].bitcast(F32R))
    ew_t = pers.tile([SS, D], F32R, tag="ew")
    nc.sync.dma_start(ew_t[:], t["embed_s_W"][:].bitcast(F32R))
    ew3_t = pers.tile([3, D], F32R, tag="ew3")
    nc.sync.dma_start(ew3_t[:], t["p_w3"][:].bitcast(F32R))

    for _rep in range(_REPS):
      _emit_body(nc, tc, t, xt, st_t, ras_t, ew_t, ew3_t, ones_r, eps_t,
                 maskv, dynvals, _rep)


def _emit_body(nc, tc, t, xt, st_t, ras_t, ew_t, ew3_t, ones_r, eps_t,
               maskv, dynvals, rep):
    with ExitStack() as ex:
        ep = ex.enter_context(
            tc.tile_pool(name=f"eps{rep}", bufs=2, space="PSUM"))
        for dt in range(DT):
            p = ep.tile([128, S], F32, tag="e")
            for c0, cw in _nchunks(S):
                nc.tensor.matmul(p[:, c0:c0 + cw],
                                 ew_t[:, dt * 128:(dt + 1) * 128],
                                 st_t[:, c0:c0 + cw], start=True, stop=False)
                nc.tensor.matmul(p[:, c0:c0 + cw],
                                 ew3_t[:, dt * 128:(dt + 1) * 128],
                                 ras_t[:, c0:c0 + cw], start=False, stop=True)
            nc.vector.tensor_copy(xt[dt][:], p[:])

    # ---- layers ----
    for l in range(NL):
        _layer(nc, tc, t, l + rep * NL, xt, ones_r, eps_t, maskv, dynvals)

    # ---- head ----
    with ExitStack() as ex:
        hp = ex.enter_context(tc.tile_pool(name=f"hp{rep}", bufs=1))
        pp = ex.enter_context(
            tc.tile_pool(name=f"hpp{rep}", bufs=1, space="PSUM"))
        awt = hp.tile([128, DT * A], F32R, tag="awt")
        nc.sync.dma_start(awt[:], t["p_aw"][:].bitcast(F32R))
        abt = hp.tile([A, 1], F32, tag="abt")
        nc.sync.dma_start(abt[:], t["p_ab"][:])
        ph = pp.tile([A, L], F32, tag="ph")
        for dt in range(DT):
            nc.tensor.matmul(ph[:], awt[:, dt * A:(dt + 1) * A],
                             xt[dt][:, 256:768],
                             start=(dt == 0), stop=(dt == DT - 1))
        hsb = hp.tile([A, L], F32, tag="hsb")
        nc.scalar.add(hsb[:], ph[:], abt[:, 0:1])
        nc.sync.dma_start(t["head"][:], hsb[:])


def _layer(nc, tc, t, l, xt, ones_r, eps_t, maskv, dynvals):
    lw = l % NL   # weight index (reps reuse weights)
    # ======== attention + Wo + LN1 (wo/o8 pools scoped here) ========
    with ExitStack() as lx:
        wp = lx.enter_context(tc.tile_pool(name=f"w{l}", bufs=1))
        wo_t = wp.tile([128, DT * D], F32R, tag="wo")
        nc.sync.dma_start(wo_t[:], t["p_wo"][lw].bitcast(F32R))
        bo_t = wp.tile([128, DT], F32, tag="bo")
        nc.sync.dma_start(bo_t[:], t["p_bo"][lw])
        g1_t = wp.tile([128, DT], F32, tag="g1")
        nc.sync.dma_start(g1_t[:], t["p_g1"][lw])
        c1_t = wp.tile([128, DT], F32, tag="c1")
        nc.sync.dma_start(c1_t[:], t["p_c1"][lw])
        o8 = [wp.tile([128, T], F32R, tag=f"o8_{i}", name=f"o8_{l}_{i}")
              for i in range(DT)]

        with ExitStack() as ax:
            aw = ax.enter_context(tc.tile_pool(name=f"aw{l}", bufs=1))
            wq_t = aw.tile([128, H * DH // 2], F32R, tag="wq")
            nc.sync.dma_start(wq_t[:], t["p_wq"][lw].bitcast(F32R))
            wk_t = aw.tile([128, H * DH // 2], F32R, tag="wk")
            nc.sync.dma_start(wk_t[:], t["p_wk"][lw].bitcast(F32R))
            wv_t = aw.tile([128, H * DH // 2], F32R, tag="wv")
            nc.sync.dma_start(wv_t[:], t["p_wv"][lw].bitcast(F32R))
            bq_t = aw.tile([DH, H], F32, tag="bq")
            nc.sync.dma_start(bq_t[:], t["p_bq"][lw])
            bk_t = aw.tile([DH, H], F32, tag="bk")
            nc.sync.dma_start(bk_t[:], t["p_bk"][lw])
            bvr = aw.tile([1, D], F32, tag="bvr")
            nc.sync.dma_start(bvr[:], t["p_bv"][lw])
            bv_bc = aw.tile([128, D], F32, tag="bvbc")
            nc.gpsimd.partition_broadcast(bv_bc[:], bvr[0:1, :])

            pp = ax.enter_context(tc.tile_pool(name=f"pp{l}", bufs=2,
                                               space="PSUM"))
            pss = ax.enter_context(tc.tile_pool(name=f"pss{l}", bufs=2,
                                                space="PSUM"))
            pso = ax.enter_context(tc.tile_pool(name=f"pso{l}", bufs=1,
                                                space="PSUM"))
            sp = ax.enter_context(tc.tile_pool(name=f"sp{l}", bufs=3))
            epl = ax.enter_context(tc.tile_pool(name=f"ep{l}", bufs=4))
            npl = ax.enter_context(tc.tile_pool(name=f"np{l}", bufs=2))
            adt = BF16 if _ATT_BF16 else F32R
            amax = 512
            for h in range(H):
                xi, pr = h // 2, (h % 2) * 64
                xh = xt[xi][pr:pr + 64, :]
                ws = slice(xi * DH, (xi + 1) * DH)
                wrow = slice(pr, pr + 64)

                q_sb = sp.tile([DH, T], adt, tag="q")
                for c0, cw in _nchunks(T):
                    pq = pp.tile([128, 512], F32, tag="pp")
                    nc.tensor.matmul(pq[0:DH, 0:cw], wq_t[wrow, ws],
                                     xh[:, c0:c0 + cw], start=True, stop=True)
                    nc.scalar.add(q_sb[:, c0:c0 + cw], pq[0:DH, 0:cw],
                                  bq_t[:, h:h + 1])
                k_sb = sp.tile([DH, S], adt, tag="k")
                for c0, cw in _nchunks(S):
                    pk = pp.tile([128, 512], F32, tag="pp")
                    nc.tensor.matmul(pk[0:DH, 0:cw], wk_t[wrow, ws],
                                     xh[:, c0:c0 + cw], start=True, stop=True)
                    nc.scalar.add(k_sb[:, c0:c0 + cw], pk[0:DH, 0:cw],
                                  bk_t[:, h:h + 1])
                v_sb = sp.tile([128, 65 * KT], adt, tag="v")
                if _ATT_BF16:
                    # bf16 memset value unsupported; write 1.0 bit pattern
                    nc.vector.memset(v_sb[:].bitcast(mybir.dt.uint16), 0x3F80)
                else:
                    nc.vector.memset(v_sb[:].bitcast(F32), 1.0)
                for j in range(KT):
                    pv = pp.tile([128, 512], F32, tag="pp")
                    nc.tensor.matmul(pv[:, 0:DH], xh[:, j * 128:(j + 1) * 128],
                                     wv_t[wrow, ws], start=True, stop=True)
                    nc.vector.tensor_add(v_sb[:, j * 65:j * 65 + DH],
                                         pv[:, 0:DH], bv_bc[:, h * DH:(h + 1) * DH])

                po = pso.tile([65, T], F32, tag="po")
                for j in range(KT):
                    ps = pss.tile([128, T], F32, tag="ps")
                    for c0, cw in _nchunks(T, amax):
                        nc.tensor.matmul(ps[:, c0:c0 + cw],
                                         k_sb[:, j * 128:(j + 1) * 128],
                                         q_sb[:, c0:c0 + cw],
                                         start=True, stop=True)
                    ex_t = epl.tile([128, T], adt, tag="ex")
                    nc.scalar.activation(ex_t[:], ps[:], AF.Exp)
                    nc.vector.tensor_scalar_mul(
                        ex_t[:, T - 1:T], ex_t[:, T - 1:T],
                        maskv[:, j:j + 1])
                    for c0, cw in _nchunks(T, amax):
                        nc.tensor.matmul(po[:, c0:c0 + cw],
                                         v_sb[:, j * 65:(j + 1) * 65],
                                         ex_t[:, c0:c0 + cw],
                                         start=(j == 0), stop=(j == KT - 1))
                # normalize: rows 0..63 are o^T_h, row 64 is sum(exp)
                orw = npl.tile([65, T], F32, tag="orw")
                nc.vector.tensor_copy(orw[0:64, :], po[0:64, :])
                nc.vector.tensor_copy(orw[64:65, :], po[64:65, :])
                rcp = npl.tile([1, T], F32, tag="rcp")
                nc.vector.reciprocal(rcp[0:1, :], orw[64:65, :])
                bc = npl.tile([64, T], F32, tag="bc")
                nc.gpsimd.partition_broadcast(bc[:], rcp[0:1, :])
                nc.vector.scalar_tensor_tensor(
                    o8[xi][pr:pr + 64, :], orw[0:64, :],
                    1.0 / 32.0, bc[:], ALU.mult, ALU.mult)

        # ============ Wo + residual + LN1 ============
        with ExitStack() as ax:
            rp = ax.enter_context(tc.tile_pool(name=f"r{l}a", bufs=1))
            r1 = [rp.tile([128, T], F32R, tag=f"r1_{i}", name=f"r1_{l}_{i}")
                  for i in range(DT)]
            pw = ax.enter_context(tc.tile_pool(name=f"pw{l}", bufs=2,
                                               space="PSUM"))
            tp = ax.enter_context(tc.tile_pool(name=f"tp{l}", bufs=2))
            for dc in range(DT):
                p = pw.tile([128, T], F32, tag="pw")
                for dt in range(DT):
                    for c0, cw in _nchunks(T):
                        nc.tensor.matmul(
                            p[:, c0:c0 + cw],
                            wo_t[:, dt * D + dc * 128:dt * D + (dc + 1) * 128],
                            o8[dt][:, c0:c0 + cw],
                            start=(dt == 0), stop=(dt == DT - 1))
                tmp = tp.tile([128, T], F32, tag="tmp")
                nc.scalar.add(tmp[:], p[:], bo_t[:, dc:dc + 1])
                nc.vector.tensor_add(r1[dc][:], tmp[:],
                                     xt[dc][:, 0:T].bitcast(F32))
            _layernorm(nc, tc, r1, xt, ones_r, eps_t, g1_t, c1_t, l, 1)

    # ======== FFN + LN2 (attention pools closed) ========
    with ExitStack() as lx:
        fp = lx.enter_context(tc.tile_pool(name=f"f{l}", bufs=1))
        b1_t = fp.tile([128, FC], F32, tag="b1")
        nc.sync.dma_start(b1_t[:], t["p_b1"][lw])
        b2_t = fp.tile([128, DT], F32, tag="b2")
        nc.sync.dma_start(b2_t[:], t["p_b2"][lw])
        g2_t = fp.tile([128, DT], F32, tag="g2")
        nc.sync.dma_start(g2_t[:], t["p_g2"][lw])
        c2_t = fp.tile([128, DT], F32, tag="c2")
        nc.sync.dma_start(c2_t[:], t["p_c2"][lw])
        o2 = [fp.tile([128, T], F32, tag=f"o2_{i}", name=f"o2_{l}_{i}")
              for i in range(DT)]
        for half in range(2):
            with ExitStack() as fx:
                h1p = fx.enter_context(tc.tile_pool(name=f"h1p{l}_{half}",
                                                    bufs=1))
                w1p = fx.enter_context(tc.tile_pool(name=f"w1p{l}_{half}",
                                                    bufs=3))
                h1 = []
                with ExitStack() as px:
                    p1p = px.enter_context(
                        tc.tile_pool(name=f"p1{l}_{half}", bufs=4,
                                     space="PSUM"))
                    for fi in range(16):
                        fc = half * 16 + fi
                        w1c = w1p.tile([128, DT * 128], F32R, tag="w1c")
                        nc.sync.dma_start(w1c[:],
                                          t["p_w1"][lw, fc].bitcast(F32R))
                        p = p1p.tile([128, T], F32, tag="p1")
                        for dt in range(DT):
                            for c0, cw in _nchunks(T):
                                nc.tensor.matmul(
                                    p[:, c0:c0 + cw],
                                    w1c[:, dt * 128:(dt + 1) * 128],
                                    xt[dt][:, c0:c0 + cw],
                                    start=(dt == 0), stop=(dt == DT - 1))
                        ht = h1p.tile([128, T], F32R, tag=f"h1_{fi}",
                                      name=f"h1_{l}_{half}_{fi}")
                        nc.scalar.activation(ht[:], p[:], AF.Relu,
                                             bias=b1_t[:, fc:fc + 1])
                        h1.append(ht)
                with ExitStack() as px:
                    p2p = px.enter_context(
                        tc.tile_pool(name=f"p2{l}_{half}", bufs=3,
                                     space="PSUM"))
                    w2p = px.enter_context(
                        tc.tile_pool(name=f"w2p{l}_{half}", bufs=3))
                    for dc in range(DT):
                        w2c = w2p.tile([128, 16 * 128], F32R, tag="w2c")
                        nc.sync.dma_start(w2c[:],
                                          t["p_w2"][lw, dc, half].bitcast(F32R))
                        p = p2p.tile([128, T], F32, tag="p2")
                        for fi in range(16):
                            for c0, cw in _nchunks(T):
                                nc.tensor.matmul(
                                    p[:, c0:c0 + cw],
                                    w2c[:, fi * 128:(fi + 1) * 128],
                                    h1[fi][:, c0:c0 + cw],
                                    start=(fi == 0), stop=(fi == 15))
                        if half == 0:
                            nc.vector.tensor_copy(o2[dc][:], p[:])
                        else:
                            nc.vector.tensor_add(o2[dc][:], o2[dc][:], p[:])

        # r2 = y + ffn + b2
        with ExitStack() as ax:
            rp = ax.enter_context(tc.tile_pool(name=f"r{l}b", bufs=1))
            r2 = [rp.tile([128, T], F32R, tag=f"r2_{i}", name=f"r2_{l}_{i}")
                  for i in range(DT)]
            tp = ax.enter_context(tc.tile_pool(name=f"tq{l}", bufs=2))
            for dc in range(DT):
                tmp = tp.tile([128, T], F32, tag="tmp")
                nc.scalar.add(tmp[:], o2[dc][:], b2_t[:, dc:dc + 1])
                nc.vector.tensor_add(r2[dc][:], tmp[:],
                                     xt[dc][:, 0:T].bitcast(F32))
            _layernorm(nc, tc, r2, xt, ones_r, eps_t, g2_t, c2_t, l, 2)

    # ======== exchange halves (not after last layer) ========
    if (l % NL) < NL - 1 and not _NO_CC:
        for dt in range(DT):
            nc.sync.dma_start(t["ag_in"][dt * 128:(dt + 1) * 128, :],
                              xt[dt][:, 0:T].bitcast(F32))
        nc.gpsimd.collective_compute(
            "AllGather", ALU.bypass, replica_groups=GROUPS,
            ins=[t["ag_in"][:]], outs=[t["ag_out"][:]])
        for dt in range(DT):
            src = bass.AP(t["ag_out"], dynvals[dt],
                          [[T, 128], [1, T]]).bitcast(F32R)
            nc.gpsimd.dma_start(xt[dt][:, T:S], src)


def _layernorm(nc, tc, rin, xt, ones_r, eps_t, g_t, c_t, l, which):
    """LN over features (partition dim), writes result into xt[:, 0:T]."""
    with ExitStack() as ax:
        pst = ax.enter_context(
            tc.tile_pool(name=f"pst{l}_{which}", bufs=1, space="PSUM"))
        sq = ax.enter_context(tc.tile_pool(name=f"sq{l}_{which}", bufs=2))
        st = ax.enter_context(tc.tile_pool(name=f"st{l}_{which}", bufs=1))
        pm = pst.tile([1, T], F32, tag="pm")
        pq = pst.tile([1, T], F32, tag="pq")
        for dt in range(DT):
            s = sq.tile([128, T], F32R, tag="sq")
            nc.vector.tensor_mul(s[:], rin[dt][:].bitcast(F32),
                                 rin[dt][:].bitcast(F32))
            for c0, cw in _nchunks(T):
                nc.tensor.matmul(pm[:, c0:c0 + cw], ones_r[:],
                                 rin[dt][:, c0:c0 + cw],
                                 start=(dt == 0), stop=(dt == DT - 1))
                nc.tensor.matmul(pq[:, c0:c0 + cw], ones_r[:],
                                 s[:, c0:c0 + cw],
                                 start=(dt == 0), stop=(dt == DT - 1))
        mu = st.tile([1, T], F32, tag="mu")
        nc.scalar.mul(mu[0:1, :], pm[0:1, :], 1.0 / D)
        t1 = st.tile([1, T], F32, tag="t1")
        nc.scalar.mul(t1[0:1, :], pq[0:1, :], 1.0 / D)
        msq = st.tile([1, T], F32, tag="msq")
        nc.vector.tensor_mul(msq[0:1, :], mu[0:1, :], mu[0:1, :])
        var = st.tile([1, T], F32, tag="var")
        nc.vector.tensor_sub(var[0:1, :], t1[0:1, :], msq[0:1, :])
        sd = st.tile([1, T], F32, tag="sd")
        nc.scalar.activation(sd[0:1, :], var[0:1, :], AF.Sqrt,
                             bias=eps_t[0:1, 0:1])
        rstd = st.tile([1, T], F32, tag="rstd")
        nc.vector.reciprocal(rstd[0:1, :], sd[0:1, :])
        nb = st.tile([1, T], F32, tag="nb")
        nc.vector.scalar_tensor_tensor(nb[0:1, :], mu[0:1, :], -1.0,
                                       rstd[0:1, :], ALU.mult, ALU.mult)
        a_bc = st.tile([128, T], F32, tag="abc")
        nc.gpsimd.partition_broadcast(a_bc[:], rstd[0:1, :])
        b_bc = st.tile([128, T], F32, tag="bbc")
        nc.gpsimd.partition_broadcast(b_bc[:], nb[0:1, :])
        tmp2 = ax.enter_context(tc.tile_pool(name=f"tl{l}_{which}", bufs=2))
        for dt in range(DT):
            u = tmp2.tile([128, T], F32, tag="u")
            nc.vector.tensor_mul(u[:], rin[dt][:].bitcast(F32), a_bc[:])
            w = tmp2.tile([128, T], F32, tag="w")
            nc.vector.tensor_add(w[:], u[:], b_bc[:])
            nc.scalar.activation(xt[dt][:, 0:T], w[:], AF.Identity,
                                 bias=c_t[:, dt:dt + 1],
                                 scale=g_t[:, dt:dt + 1])


def _build(no_cc=False, reps=1, att_bf16=True):
    key = ("nc", no_cc, reps, att_bf16)
    if key in _CACHE:
        return _CACHE[key]
    global _NO_CC, _REPS, _ATT_BF16
    _NO_CC = no_cc
    _REPS = reps
    _ATT_BF16 = att_bf16
    nc = bacc.Bacc("TRN2", target_bir_lowering=False, debug=False,
                   num_devices=NC)
    t = {}

    def inp(name, shape, dtype=F32):
        t[name] = nc.dram_tensor(name, shape, dtype, kind="ExternalInput")
        return t[name]

    inp("p_state", [SS, S])
    inp("p_ras", [3, S])
    inp("embed_s_W", [SS, D])
    inp("p_w3", [3, D])
    inp("p_wq", [NL, 128, H * DH // 2])
    inp("p_wk", [NL, 128, H * DH // 2])
    inp("p_wv", [NL, 128, H * DH // 2])
    inp("p_bq", [NL, DH, H])
    inp("p_bk", [NL, DH, H])
    inp("p_bv", [NL, 1, D])
    inp("p_wo", [NL, 128, DT * D])
    inp("p_bo", [NL, 128, DT])
    inp("p_g1", [NL, 128, DT])
    inp("p_c1", [NL, 128, DT])
    inp("p_g2", [NL, 128, DT])
    inp("p_c2", [NL, 128, DT])
    inp("p_b1", [NL, 128, FC])
    inp("p_b2", [NL, 128, DT])
    inp("p_w1", [NL, FC, 128, DT * 128])
    inp("p_w2", [NL, DT, 2, 128, 16 * 128])
    inp("p_aw", [128, DT * A])
    inp("p_ab", [A, 1])
    inp("p_maskv", [128, KT])
    inp("p_offs", [1, DT], U32)
    t["ag_in"] = nc.dram_tensor("ag_in", [D, T], F32)
    t["ag_out"] = nc.dram_tensor("ag_out", [2, D, T], F32)
    t["head"] = nc.dram_tensor("head", [A, L], F32, kind="ExternalOutput")

    with tile.TileContext(nc) as tc, ExitStack() as ctx:
        _emit(nc, tc, ctx, t)
    nc.compile()
    _CACHE[key] = (nc, t)
    return nc, t


def _make_in_maps(inputs):
    f32 = lambda x: np.ascontiguousarray(np.asarray(x), dtype=np.float32)
    reward = f32(inputs["reward"])
    state = f32(inputs["state"])
    action = f32(inputs["action"])
    timestep = f32(inputs["timestep"])
    Wq, bq = f32(inputs["Wq"]), f32(inputs["bq"])
    Wk, bk = f32(inputs["Wk"]), f32(inputs["bk"])
    Wv, bv = f32(inputs["Wv"]), f32(inputs["bv"])
    Wo, bo = f32(inputs["Wo"]), f32(inputs["bo"])
    W1, b1 = f32(inputs["W1"]), f32(inputs["b1"])
    W2, b2 = f32(inputs["W2"]), f32(inputs["b2"])

    shared = {
        "embed_s_W": f32(inputs["embed_s_W"]),
        "p_w3": np.ascontiguousarray(np.stack([
            f32(inputs["embed_R_W"])[0],
            f32(inputs["embed_a_W"])[0],
            f32(inputs["embed_t_W"])[0]])),
        "p_wq": np.ascontiguousarray(
            Wq.transpose(0, 2, 1, 3).reshape(NL, DH, H // 2, 2, DH)
            .transpose(0, 3, 1, 2, 4).reshape(NL, 128, H * DH // 2)),
        "p_wk": np.ascontiguousarray(
            Wk.transpose(0, 2, 1, 3).reshape(NL, DH, H // 2, 2, DH)
            .transpose(0, 3, 1, 2, 4).reshape(NL, 128, H * DH // 2)),
        "p_wv": np.ascontiguousarray(
            Wv.transpose(0, 2, 1, 3).reshape(NL, DH, H // 2, 2, DH)
            .transpose(0, 3, 1, 2, 4).reshape(NL, 128, H * DH // 2)),
        "p_bq": np.ascontiguousarray(bq.transpose(0, 2, 1)),
        "p_bk": np.ascontiguousarray(bk.transpose(0, 2, 1)),
        "p_bv": np.ascontiguousarray(bv.reshape(NL, 1, D)),
        "p_wo": np.ascontiguousarray(
            Wo.reshape(NL, DT, 128, D).transpose(0, 2, 1, 3)
            .reshape(NL, 128, DT * D)),
        "p_bo": np.ascontiguousarray(
            bo.reshape(NL, DT, 128).transpose(0, 2, 1)),
        "p_g1": np.ascontiguousarray(
            f32(inputs["ln1_g"]).reshape(NL, DT, 128).transpose(0, 2, 1)),
        "p_c1": np.ascontiguousarray(
            f32(inputs["ln1_b"]).reshape(NL, DT, 128).transpose(0, 2, 1)),
        "p_g2": np.ascontiguousarray(
            f32(inputs["ln2_g"]).reshape(NL, DT, 128).transpose(0, 2, 1)),
        "p_c2": np.ascontiguousarray(
            f32(inputs["ln2_b"]).reshape(NL, DT, 128).transpose(0, 2, 1)),
        "p_b1": np.ascontiguousarray(
            b1.reshape(NL, FC, 128).transpose(0, 2, 1)),
        "p_b2": np.ascontiguousarray(
            b2.reshape(NL, DT, 128).transpose(0, 2, 1)),
        "p_w1": np.ascontiguousarray(
            W1.reshape(NL, DT, 128, FC, 128).transpose(0, 3, 2, 1, 4)
            .reshape(NL, FC, 128, DT * 128)),
        "p_w2": np.ascontiguousarray(
            W2.reshape(NL, 2, 16, 128, DT, 128).transpose(0, 4, 1, 3, 2, 5)
            .reshape(NL, DT, 2, 128, 16 * 128)),
        "p_aw": np.ascontiguousarray(
            f32(inputs["action_W"]).reshape(DT, 128, A).transpose(1, 0, 2)
            .reshape(128, DT * A)),
        "p_ab": np.ascontiguousarray(f32(inputs["action_b"]).reshape(A, 1)),
    }

    in_maps = []
    for c in range(NC):
        b, half = divmod(c, 2)
        g = (np.arange(S) + half * T) % S          # local col -> global token
        st_loc = np.zeros((SS, S), np.float32)
        m = (g >= 512) & (g < 1024)
        st_loc[:, m] = state[b, g[m] - 512, :].T
        ras = np.zeros((3, S), np.float32)
        m0 = g < 512
        ras[0, m0] = reward[b, g[m0], 0]
        m2 = g >= 1024
        ras[1, m2] = action[b, g[m2] - 1024, 0]
        ras[2, :] = np.sign(timestep[b, g % 512, 0] / float(D))
        maskv = np.ones((128, KT), np.float32)
        if half == 1:
            maskv[:, 2:6] = 0.0
        otherp = 1 - half
        offs = np.array([[otherp * D * T + dt * 128 * T for dt in range(DT)]],
                        dtype=np.uint32)
        m = dict(shared)
        m["p_state"] = st_loc
        m["p_ras"] = ras
        m["p_maskv"] = maskv
        m["p_offs"] = offs
        in_maps.append(m)
    return in_maps


def kernel(**inputs):
    nc, _ = _build()
    in_maps = _make_in_maps(inputs)
    res = run_bass_kernel_spmd(nc, in_maps, list(range(NC)))
    out = np.stack([res.results[2 * b + 1]["head"].T for b in range(B)])
    return out.astype(np.float32)


if __name__ == "__main__":
    rng = np.random.default_rng(0)
    demo = {
        "reward": rng.standard_normal((B, L, 1)),
        "state": rng.standard_normal((B, L, SS)),
        "action": rng.standard_normal((B, L, 1)),
        "timestep": rng.random((B, L, 1)),
        "embed_R_W": rng.standard_normal((1, D)) * 0.02,
        "embed_s_W": rng.standard_normal((SS, D)) * 0.02,
        "embed_a_W": rng.standard_normal((1, D)) * 0.02,
        "embed_t_W": rng.standard_normal((1, D)) * 0.02,
        "Wq": rng.standard_normal((NL, H, DH, DH)) * 0.02,
        "bq": np.zeros((NL, H, DH)),
        "Wk": rng.standard_normal((NL, H, DH, DH)) * 0.02,
        "bk": np.zeros((NL, H, DH)),
        "Wv": rng.standard_normal((NL, H, DH, DH)) * 0.02,
        "bv": np.zeros((NL, H, DH)),
        "Wo": rng.standard_normal((NL, D, D)) * 0.02,
        "bo": np.zeros((NL, D)),
        "ln1_g": np.ones((NL, D)),
        "ln1_b": np.zeros((NL, D)),
        "W1": rng.standard_normal((NL, D, FF)) * 0.02,
        "b1": np.zeros((NL, FF)),
        "W2": rng.standard_normal((NL, FF, D)) * 0.02,
        "b2": np.zeros((NL, D)),
        "ln2_g": np.ones((NL, D)),
        "ln2_b": np.zeros((NL, D)),
        "action_W": rng.standard_normal((D, A)) * 0.02,
        "action_b": np.zeros((A,)),
    }
    out = kernel(**demo)
    print("out", out.shape, out.dtype, float(np.abs(out).max()))



# revision 9
# speedup vs baseline: 1.2497x; 1.2497x over previous
"""DecisionTransformer Trainium2 kernel: 8 NeuronCores, SPMD.

Sharding: core c handles batch c//2 and sequence-half c%2 (768 of 1536
tokens). Within a pair of cores the full key/value sequence is rebuilt
each layer via a pair AllGather (bf16). Residual stream is kept
feature-major (X^T) in SBUF as bf16; per-head QKV projections run as
head-pair block-diagonal 128x128 matmuls; scores for the two heads of a
pair run concurrently in disjoint PE row groups.
"""
import numpy as np
from contextlib import ExitStack

import concourse.bass as bass
import concourse.tile as tile
from concourse import bacc, mybir
from concourse.bass_utils import run_bass_kernel_spmd

B, L, D, H, FF, NL, SS, A = 4, 512, 1024, 16, 4096, 4, 128, 16
DH = 64
S = 3 * L            # 1536 tokens
T = S // 2           # 768 own tokens per core
NC = 8
NP = H // 2          # 8 head pairs
KT = S // 128        # 12 key tiles
DT = D // 128        # 8 feature tiles
FC = FF // 128       # 32 ffn tiles
F32 = mybir.dt.float32
BF16 = mybir.dt.bfloat16
U32 = mybir.dt.uint32
AF = mybir.ActivationFunctionType
ALU = mybir.AluOpType
GROUPS = [[0, 1], [2, 3], [4, 5], [6, 7]]

_CACHE = {}
_NO_CC = False
_REPS = 1


def _nchunks(n, c=512):
    out = []
    o = 0
    while o < n:
        w = min(c, n - o)
        out.append((o, w))
        o += w
    return out


def _emit(nc, tc, ctx, t):
    pers = ctx.enter_context(tc.tile_pool(name="pers", bufs=1))

    # ---- persistent small tiles ----
    ones_b = pers.tile([128, 1], BF16, tag="ones")
    nc.vector.memset(ones_b[:].bitcast(mybir.dt.uint16), 0x3F80)
    ones_r = pers.tile([128, 1], mybir.dt.float32r, tag="onesr")
    nc.vector.memset(ones_r[:].bitcast(F32), 1.0)
    eps_t = pers.tile([1, 1], F32, tag="eps")
    nc.vector.memset(eps_t[:], 1e-5)
    maskv = pers.tile([128, KT], F32, tag="maskv")
    nc.sync.dma_start(maskv[:], t["p_maskv"][:])
    offs_t = pers.tile([1, DT], U32, tag="offs")
    nc.sync.dma_start(offs_t[:], t["p_offs"][:])

    # dynamic offsets for reading the peer half of ag_out (per-core data)
    POOL = mybir.EngineType.Pool
    dynvals = []
    for dt in range(DT):
        reg = nc.alloc_registers(engines=[POOL])
        nc.gpsimd.reg_load(reg, offs_t[0:1, dt:dt + 1])
        dynvals.append(nc.snap(reg, donate=True, min_val=0, max_val=2 * D * T))

    # ---- residual stream X^T, 8 tiles [128, S] bf16 ----
    xt = [pers.tile([128, S], BF16, tag=f"xt{i}", name=f"xt{i}")
          for i in range(DT)]
    # precise f32 residual for the own half (peer half is K/V-only)
    xp = [pers.tile([128, T], F32, tag=f"xp{i}", name=f"xp{i}")
          for i in range(DT)]

    # ---- embedding inputs ----
    st_t = pers.tile([SS, S], BF16, tag="st")
    nc.sync.dma_start(st_t[:], t["p_state"][:])
    ras_t = pers.tile([3, S], BF16, tag="ras")
    nc.sync.dma_start(ras_t[:], t["p_ras"][:])
    ew_t = pers.tile([SS, D], BF16, tag="ew")
    nc.sync.dma_start(ew_t[:], t["embed_s_W"][:])
    ew3_t = pers.tile([3, D], BF16, tag="ew3")
    nc.sync.dma_start(ew3_t[:], t["p_w3"][:])

    for _rep in range(_REPS):
        _emit_body(nc, tc, t, xt, xp, st_t, ras_t, ew_t, ew3_t, ones_b,
                   ones_r, eps_t, maskv, dynvals, _rep)


def _emit_body(nc, tc, t, xt, xp, st_t, ras_t, ew_t, ew3_t, ones_b,
               ones_r, eps_t, maskv, dynvals, rep):
    with ExitStack() as ex:
        ep = ex.enter_context(
            tc.tile_pool(name=f"eps{rep}", bufs=2, space="PSUM"))
        for dt in range(DT):
            p = ep.tile([128, S], F32, tag="e")
            for c0, cw in _nchunks(S):
                nc.tensor.matmul(p[:, c0:c0 + cw],
                                 ew_t[:, dt * 128:(dt + 1) * 128],
                                 st_t[:, c0:c0 + cw], start=True, stop=False)
                nc.tensor.matmul(p[:, c0:c0 + cw],
                                 ew3_t[:, dt * 128:(dt + 1) * 128],
                                 ras_t[:, c0:c0 + cw], start=False, stop=True)
            nc.vector.tensor_copy(xt[dt][:], p[:])
            nc.vector.tensor_copy(xp[dt][:], p[:, 0:T])

    # ---- layers ----
    for l in range(NL):
        _layer(nc, tc, t, l + rep * NL, xt, xp, ones_b, ones_r, eps_t,
               maskv, dynvals)

    # ---- head ----
    with ExitStack() as ex:
        hp = ex.enter_context(tc.tile_pool(name=f"hp{rep}", bufs=1))
        pp = ex.enter_context(
            tc.tile_pool(name=f"hpp{rep}", bufs=1, space="PSUM"))
        awt = hp.tile([128, DT * A], BF16, tag="awt")
        nc.sync.dma_start(awt[:], t["p_aw"][:])
        abt = hp.tile([A, 1], F32, tag="abt")
        nc.sync.dma_start(abt[:], t["p_ab"][:])
        ph = pp.tile([A, L], F32, tag="ph")
        for dt in range(DT):
            nc.tensor.matmul(ph[:], awt[:, dt * A:(dt + 1) * A],
                             xt[dt][:, 256:768],
                             start=(dt == 0), stop=(dt == DT - 1))
        hsb = hp.tile([A, L], F32, tag="hsb")
        nc.scalar.add(hsb[:], ph[:], abt[:, 0:1])
        nc.sync.dma_start(t["head"][:], hsb[:])


def _ln_apply(nc, st, tp, pm, pq2, eps_t, rv, xp, xt, g_t, c_t, dtc):
    """LN chain + apply, split into two column groups so the scalar
    chain of group B pipelines under group A's apply, with the
    elementwise work spread over DVE (group A) and Pool (group B)."""
    mu = st.tile([1, T], F32, tag="mu")
    tmp1 = st.tile([1, T], F32, tag="t1")
    var = st.tile([1, T], F32, tag="var")
    sd = st.tile([1, T], F32, tag="sd")
    rstd = st.tile([1, T], F32, tag="rstd")
    nb = st.tile([1, T], F32, tag="nb")
    bc_a = st.tile([128, T], F32, tag="bca")
    bc_b = st.tile([128, T], F32, tag="bcb")
    for gi, (c0, cw) in enumerate(_nchunks(T)):
        cs = slice(c0, c0 + cw)
        nc.scalar.mul(mu[0:1, cs], pm[0:1, cs], 1.0 / D)
        nc.vector.tensor_mul(tmp1[0:1, cs], mu[0:1, cs], mu[0:1, cs])
        nc.vector.scalar_tensor_tensor(var[0:1, cs], pq2[0:1, cs], 1.0 / D,
                                       tmp1[0:1, cs], ALU.mult, ALU.subtract)
        nc.scalar.activation(sd[0:1, cs], var[0:1, cs], AF.Sqrt,
                             bias=eps_t[0:1, 0:1])
        nc.vector.reciprocal(rstd[0:1, cs], sd[0:1, cs])
        nc.vector.scalar_tensor_tensor(nb[0:1, cs], mu[0:1, cs], -1.0,
                                       rstd[0:1, cs], ALU.mult, ALU.mult)
        nc.gpsimd.partition_broadcast(bc_a[:, cs], rstd[0:1, cs])
        nc.gpsimd.partition_broadcast(bc_b[:, cs], nb[0:1, cs])
        for dt in range(DT):
            u = tp.tile([128, 512], F32, tag="u")
            w = tp.tile([128, 512], F32, tag="w")
            eng = nc.vector if gi == 0 else nc.gpsimd
            eng.tensor_mul(u[:, 0:cw], rv[dt][:, cs].bitcast(F32),
                           bc_a[:, cs])
            eng.tensor_add(w[:, 0:cw], u[:, 0:cw], bc_b[:, cs])
            nc.scalar.activation(xp[dt][:, cs], w[:, 0:cw], AF.Identity,
                                 bias=c_t[:, dtc + dt:dtc + dt + 1],
                                 scale=g_t[:, dtc + dt:dtc + dt + 1])
            ceng = nc.gpsimd if gi == 0 else nc.vector
            ceng.tensor_copy(xt[dt][:, cs], xp[dt][:, cs])


def _qkv(nc, pp, xt, wqkv, bqk_t, bvb, q2, k2, v65, p):
    """Project pair p: q2 [128,T], k2 [128,S], v65 [128, 2*65*KT]."""
    wq = wqkv[p][:, 0:128]
    wk = wqkv[p][:, 128:256]
    wv = wqkv[p][:, 256:384]
    for c0, cw in _nchunks(T):
        pq = pp.tile([128, 512], F32, tag="pp")
        nc.tensor.matmul(pq[:, 0:cw], wq, xt[p][:, c0:c0 + cw],
                         start=True, stop=True)
        nc.vector.tensor_scalar_add(q2[:, c0:c0 + cw], pq[:, 0:cw],
                                    bqk_t[:, 2 * p:2 * p + 1])
    for c0, cw in _nchunks(S):
        pk = pp.tile([128, 512], F32, tag="pp")
        nc.tensor.matmul(pk[:, 0:cw], wk, xt[p][:, c0:c0 + cw],
                         start=True, stop=True)
        nc.vector.tensor_scalar_add(k2[:, c0:c0 + cw], pk[:, 0:cw],
                                    bqk_t[:, 2 * p + 1:2 * p + 2])
    # ones in the 65th column of every [keys, 65] block
    nc.vector.memset(v65[:].bitcast(mybir.dt.uint16), 0x3F80)
    for j in range(KT):
        pv = pp.tile([128, 512], F32, tag="pp")
        nc.tensor.matmul(pv[:, 0:128], xt[p][:, j * 128:(j + 1) * 128], wv,
                         start=True, stop=True)
        nc.vector.tensor_add(v65[:, j * 65:j * 65 + 64], pv[:, 0:64],
                             bvb[:, p * 128:p * 128 + 64])
        nc.vector.tensor_add(v65[:, 780 + j * 65:780 + j * 65 + 64],
                             pv[:, 64:128],
                             bvb[:, p * 128 + 64:p * 128 + 128])


def _layer(nc, tc, t, l, xt, xp, ones_b, ones_r, eps_t, maskv, dynvals):
    lw = l % NL   # weight index (reps reuse weights)
    # ======== attention + Wo + LN1 ========
    with ExitStack() as lx:
        wp = lx.enter_context(tc.tile_pool(name=f"w{l}", bufs=1))
        wo_t = wp.tile([128, DT * D], BF16, tag="wo")
        nc.sync.dma_start(wo_t[:], t["p_wo"][lw])
        bo_t = wp.tile([128, DT], F32, tag="bo")
        nc.sync.dma_start(bo_t[:], t["p_bo"][lw])
        g1_t = wp.tile([128, DT], F32, tag="g1")
        nc.sync.dma_start(g1_t[:], t["p_g1"][lw])
        c1_t = wp.tile([128, DT], F32, tag="c1")
        nc.sync.dma_start(c1_t[:], t["p_c1"][lw])
        o8 = [wp.tile([128, T], BF16, tag=f"o8_{i}", name=f"o8_{l}_{i}")
              for i in range(DT)]

        with ExitStack() as ax:
            aw = ax.enter_context(tc.tile_pool(name=f"aw{l}", bufs=1))
            wqkv = []
            for p in range(NP):
                w = aw.tile([128, 384], BF16, tag=f"wqkv{p}")
                nc.sync.dma_start(w[:], t["p_wqkv"][lw, p])
                wqkv.append(w)
            bqk_t = aw.tile([128, 2 * NP], F32, tag="bqk")
            nc.sync.dma_start(bqk_t[:], t["p_bqk"][lw])
            bv2_t = aw.tile([1, NP * 128], BF16, tag="bv2")
            nc.sync.dma_start(bv2_t[:], t["p_bv2"][lw])
            bvb = aw.tile([128, NP * 128], BF16, tag="bvb")
            nc.gpsimd.partition_broadcast(bvb[:], bv2_t[0:1, :])

            sp = ax.enter_context(tc.tile_pool(name=f"sp{l}", bufs=2))
            vp = ax.enter_context(tc.tile_pool(name=f"vp{l}", bufs=2))
            pp = ax.enter_context(tc.tile_pool(name=f"pp{l}", bufs=1,
                                               space="PSUM"))
            pss = ax.enter_context(tc.tile_pool(name=f"pss{l}", bufs=2,
                                                space="PSUM"))
            pov = ax.enter_context(tc.tile_pool(name=f"pov{l}", bufs=1,
                                                space="PSUM"))
            epx = ax.enter_context(tc.tile_pool(name=f"ep{l}", bufs=3))
            npx = ax.enter_context(tc.tile_pool(name=f"np{l}", bufs=2))

            q2 = sp.tile([128, T], BF16, tag="q2")
            k2 = sp.tile([128, S], BF16, tag="k2")
            v65 = vp.tile([128, 2 * 65 * KT], BF16, tag="v65")
            _qkv(nc, pp, xt, wqkv, bqk_t, bvb, q2, k2, v65, 0)

            for p in range(NP):
                po2 = pov.tile([65, 2, T], F32, tag="po2")
                for j in range(KT):
                    for h in range(2):
                        pr = h * 64
                        ps = pss.tile([128, T], F32, tag="ps")
                        for c0, cw in _nchunks(T):
                            nc.tensor.matmul(
                                ps[:, c0:c0 + cw],
                                k2[pr:pr + 64, j * 128:(j + 1) * 128],
                                q2[pr:pr + 64, c0:c0 + cw],
                                start=True, stop=True)
                        ext = epx.tile([128, T], BF16, tag="ex")
                        nc.scalar.activation(ext[:], ps[:], AF.Exp)
                        if 2 <= j < 6:
                            nc.vector.tensor_scalar_mul(
                                ext[:, T - 1:T], ext[:, T - 1:T],
                                maskv[:, j:j + 1])
                        for c0, cw in _nchunks(T):
                            nc.tensor.matmul(
                                po2[0:65, h:h + 1, c0:c0 + cw],
                                v65[:, h * 780 + j * 65:h * 780 + (j + 1) * 65],
                                ext[:, c0:c0 + cw],
                                start=(j == 0), stop=(j == KT - 1))
                if p + 1 < NP:
                    q2 = sp.tile([128, T], BF16, tag="q2")
                    k2 = sp.tile([128, S], BF16, tag="k2")
                    v65 = vp.tile([128, 2 * 65 * KT], BF16, tag="v65")
                    _qkv(nc, pp, xt, wqkv, bqk_t, bvb, q2, k2, v65, p + 1)
                # normalize both heads of the pair
                for h in range(2):
                    pr = h * 64
                    rcp = npx.tile([1, T], F32, tag="rcp")
                    nc.vector.reciprocal(rcp[0:1, :], po2[64:65, h, :])
                    bc = npx.tile([64, T], F32, tag="bc")
                    nc.gpsimd.partition_broadcast(bc[:], rcp[0:1, :])
                    nc.vector.scalar_tensor_tensor(
                        o8[p][pr:pr + 64, :], po2[0:64, h, :],
                        1.0 / 32.0, bc[:], ALU.mult, ALU.mult)

        # ============ Wo + residual + LN1 ============
        with ExitStack() as ax:
            rp = ax.enter_context(tc.tile_pool(name=f"r{l}a", bufs=1))
            r1 = [rp.tile([128, T], mybir.dt.float32r, tag=f"r1_{i}",
                           name=f"r1_{l}_{i}")
                  for i in range(DT)]
            pw = ax.enter_context(tc.tile_pool(name=f"pw{l}", bufs=2,
                                               space="PSUM"))
            pst = ax.enter_context(tc.tile_pool(name=f"pst{l}", bufs=1,
                                                space="PSUM"))
            tp = ax.enter_context(tc.tile_pool(name=f"tp{l}", bufs=2))
            sqp = ax.enter_context(tc.tile_pool(name=f"sq{l}", bufs=2))
            st = ax.enter_context(tc.tile_pool(name=f"st{l}", bufs=1))
            pm = pst.tile([1, T], F32, tag="pm")
            pq2 = pst.tile([1, T], F32, tag="pq")
            for dc in range(DT):
                p = pw.tile([128, T], F32, tag="pw")
                for dt in range(DT):
                    for c0, cw in _nchunks(T):
                        nc.tensor.matmul(
                            p[:, c0:c0 + cw],
                            wo_t[:, dt * D + dc * 128:dt * D + (dc + 1) * 128],
                            o8[dt][:, c0:c0 + cw],
                            start=(dt == 0), stop=(dt == DT - 1))
                tmp = tp.tile([128, T], BF16, tag="tmp")
                nc.scalar.add(tmp[:], p[:], bo_t[:, dc:dc + 1])
                nc.vector.tensor_add(r1[dc][:], tmp[:], xp[dc][:])
                sq = sqp.tile([128, T], BF16, tag="sq")
                nc.vector.tensor_mul(sq[:], r1[dc][:].bitcast(F32),
                                     r1[dc][:].bitcast(F32))
                for c0, cw in _nchunks(T):
                    nc.tensor.matmul(pm[:, c0:c0 + cw], ones_r[:],
                                     r1[dc][:, c0:c0 + cw],
                                     start=(dc == 0), stop=(dc == DT - 1))
                    nc.tensor.matmul(pq2[:, c0:c0 + cw], ones_b[:],
                                     sq[:, c0:c0 + cw],
                                     start=(dc == 0), stop=(dc == DT - 1))
            _ln_apply(nc, st, tp, pm, pq2, eps_t, r1, xp,
                      [x[:, 0:T] for x in xt], g1_t, c1_t, 0)

    # ======== FFN + LN2 ========
    with ExitStack() as lx:
        fp = lx.enter_context(tc.tile_pool(name=f"f{l}", bufs=1))
        b1_t = fp.tile([128, FC], F32, tag="b1")
        nc.sync.dma_start(b1_t[:], t["p_b1"][lw])
        b2_t = fp.tile([128, DT], F32, tag="b2")
        nc.sync.dma_start(b2_t[:], t["p_b2"][lw])
        g2_t = fp.tile([128, DT], F32, tag="g2")
        nc.sync.dma_start(g2_t[:], t["p_g2"][lw])
        c2_t = fp.tile([128, DT], F32, tag="c2")
        nc.sync.dma_start(c2_t[:], t["p_c2"][lw])
        h1 = [fp.tile([128, T], BF16, tag=f"h1_{i}", name=f"h1_{l}_{i}")
              for i in range(FC)]
        with ExitStack() as fx:
            w1p = fx.enter_context(tc.tile_pool(name=f"w1p{l}", bufs=4))
            p1p = fx.enter_context(tc.tile_pool(name=f"p1{l}", bufs=2,
                                                space="PSUM"))
            for fc in range(FC):
                w1c = w1p.tile([128, DT * 128], BF16, tag="w1c")
                nc.sync.dma_start(w1c[:], t["p_w1"][lw, fc])
                p = p1p.tile([128, T], F32, tag="p1")
                for dt in range(DT):
                    for c0, cw in _nchunks(T):
                        nc.tensor.matmul(
                            p[:, c0:c0 + cw],
                            w1c[:, dt * 128:(dt + 1) * 128],
                            xt[dt][:, c0:c0 + cw],
                            start=(dt == 0), stop=(dt == DT - 1))
                nc.scalar.activation(h1[fc][:], p[:], AF.Relu,
                                     bias=b1_t[:, fc:fc + 1])
        with ExitStack() as fx:
            rp = fx.enter_context(tc.tile_pool(name=f"r{l}b", bufs=1))
            r2 = [rp.tile([128, T], mybir.dt.float32r, tag=f"r2_{i}",
                           name=f"r2_{l}_{i}")
                  for i in range(DT)]
            w2p = fx.enter_context(tc.tile_pool(name=f"w2p{l}", bufs=2))
            p2p = fx.enter_context(tc.tile_pool(name=f"p2{l}", bufs=2,
                                                space="PSUM"))
            pst = fx.enter_context(tc.tile_pool(name=f"ps2{l}", bufs=1,
                                                space="PSUM"))
            tp = fx.enter_context(tc.tile_pool(name=f"tq{l}", bufs=2))
            sqp = fx.enter_context(tc.tile_pool(name=f"sr{l}", bufs=2))
            st = fx.enter_context(tc.tile_pool(name=f"s2{l}", bufs=1))
            pm = pst.tile([1, T], F32, tag="pm")
            pq2 = pst.tile([1, T], F32, tag="pq")
            for dc in range(DT):
                w2c = w2p.tile([128, FC * 128], BF16, tag="w2c")
                nc.sync.dma_start(w2c[:], t["p_w2"][lw, dc])
                p = p2p.tile([128, T], F32, tag="p2")
                for fi in range(FC):
                    for c0, cw in _nchunks(T):
                        nc.tensor.matmul(
                            p[:, c0:c0 + cw],
                            w2c[:, fi * 128:(fi + 1) * 128],
                            h1[fi][:, c0:c0 + cw],
                            start=(fi == 0), stop=(fi == FC - 1))
                tmp = tp.tile([128, T], BF16, tag="tmp")
                nc.scalar.add(tmp[:], p[:], b2_t[:, dc:dc + 1])
                nc.vector.tensor_add(r2[dc][:], tmp[:], xp[dc][:])
                sq = sqp.tile([128, T], BF16, tag="sq")
                nc.vector.tensor_mul(sq[:], r2[dc][:].bitcast(F32),
                                     r2[dc][:].bitcast(F32))
                for c0, cw in _nchunks(T):
                    nc.tensor.matmul(pm[:, c0:c0 + cw], ones_r[:],
                                     r2[dc][:, c0:c0 + cw],
                                     start=(dc == 0), stop=(dc == DT - 1))
                    nc.tensor.matmul(pq2[:, c0:c0 + cw], ones_b[:],
                                     sq[:, c0:c0 + cw],
                                     start=(dc == 0), stop=(dc == DT - 1))
            _ln_apply(nc, st, tp, pm, pq2, eps_t, r2, xp,
                      [x[:, 0:T] for x in xt], g2_t, c2_t, 0)

    # ======== exchange halves (not after last layer) ========
    if (l % NL) < NL - 1 and not _NO_CC:
        for dt in range(DT):
            nc.sync.dma_start(t["ag_in"][dt * 128:(dt + 1) * 128, :],
                              xt[dt][:, 0:T])
        nc.gpsimd.collective_compute(
            "AllGather", ALU.bypass, replica_groups=GROUPS,
            ins=[t["ag_in"][:]], outs=[t["ag_out"][:]])
        for dt in range(DT):
            src = bass.AP(t["ag_out"], dynvals[dt],
                          [[T, 128], [1, T]])
            nc.gpsimd.dma_start(xt[dt][:, T:S], src)


def _build(no_cc=False, reps=1):
    key = ("nc", no_cc, reps)
    if key in _CACHE:
        return _CACHE[key]
    global _NO_CC, _REPS
    _NO_CC = no_cc
    _REPS = reps
    nc = bacc.Bacc("TRN2", target_bir_lowering=False, debug=False,
                   num_devices=NC)
    t = {}

    def inp(name, shape, dtype=F32):
        t[name] = nc.dram_tensor(name, shape, dtype, kind="ExternalInput")
        return t[name]

    inp("p_state", [SS, S], BF16)
    inp("p_ras", [3, S], BF16)
    inp("embed_s_W", [SS, D], BF16)
    inp("p_w3", [3, D], BF16)
    inp("p_wqkv", [NL, NP, 128, 384], BF16)
    inp("p_bqk", [NL, 128, 2 * NP])
    inp("p_bv2", [NL, 1, NP * 128], BF16)
    inp("p_wo", [NL, 128, DT * D], BF16)
    inp("p_bo", [NL, 128, DT])
    inp("p_g1", [NL, 128, DT])
    inp("p_c1", [NL, 128, DT])
    inp("p_g2", [NL, 128, DT])
    inp("p_c2", [NL, 128, DT])
    inp("p_b1", [NL, 128, FC])
    inp("p_b2", [NL, 128, DT])
    inp("p_w1", [NL, FC, 128, DT * 128], BF16)
    inp("p_w2", [NL, DT, 128, FC * 128], BF16)
    inp("p_aw", [128, DT * A], BF16)
    inp("p_ab", [A, 1])
    inp("p_maskv", [128, KT])
    inp("p_offs", [1, DT], U32)
    t["ag_in"] = nc.dram_tensor("ag_in", [D, T], BF16)
    t["ag_out"] = nc.dram_tensor("ag_out", [2, D, T], BF16)
    t["head"] = nc.dram_tensor("head", [A, L], F32, kind="ExternalOutput")

    with tile.TileContext(nc) as tc, ExitStack() as ctx:
        _emit(nc, tc, ctx, t)
    nc.compile()
    _CACHE[key] = (nc, t)
    return nc, t


def _make_in_maps(inputs):
    f32 = lambda x: np.ascontiguousarray(np.asarray(x), dtype=np.float32)
    bf16 = mybir.dt.np(BF16)
    reward = f32(inputs["reward"])
    state = f32(inputs["state"])
    action = f32(inputs["action"])
    timestep = f32(inputs["timestep"])
    Wq, bq = f32(inputs["Wq"]), f32(inputs["bq"])
    Wk, bk = f32(inputs["Wk"]), f32(inputs["bk"])
    Wv, bv = f32(inputs["Wv"]), f32(inputs["bv"])
    Wo, bo = f32(inputs["Wo"]), f32(inputs["bo"])
    W1, b1 = f32(inputs["W1"]), f32(inputs["b1"])
    W2, b2 = f32(inputs["W2"]), f32(inputs["b2"])

    # head-pair block-diagonal QKV weights [NL, NP, 128, 3, 128]
    wqkv = np.zeros((NL, NP, 128, 3, 128), np.float32)
    bqk = np.zeros((NL, 128, 2 * NP), np.float32)
    bv2 = np.zeros((NL, 1, NP * 128), np.float32)
    for p in range(NP):
        for wi, Wx in enumerate((Wq, Wk, Wv)):
            wqkv[:, p, 0:64, wi, 0:64] = Wx[:, 2 * p]
            wqkv[:, p, 64:128, wi, 64:128] = Wx[:, 2 * p + 1]
        bqk[:, 0:64, 2 * p] = bq[:, 2 * p]
        bqk[:, 64:128, 2 * p] = bq[:, 2 * p + 1]
        bqk[:, 0:64, 2 * p + 1] = bk[:, 2 * p]
        bqk[:, 64:128, 2 * p + 1] = bk[:, 2 * p + 1]
        bv2[:, 0, p * 128:p * 128 + 64] = bv[:, 2 * p]
        bv2[:, 0, p * 128 + 64:p * 128 + 128] = bv[:, 2 * p + 1]

    shared = {
        "embed_s_W": f32(inputs["embed_s_W"]).astype(bf16),
        "p_w3": np.ascontiguousarray(np.stack([
            f32(inputs["embed_R_W"])[0],
            f32(inputs["embed_a_W"])[0],
            f32(inputs["embed_t_W"])[0]])).astype(bf16),
        "p_wqkv": np.ascontiguousarray(
            wqkv.reshape(NL, NP, 128, 384)).astype(bf16),
        "p_bqk": np.ascontiguousarray(bqk),
        "p_bv2": np.ascontiguousarray(bv2).astype(bf16),
        "p_wo": np.ascontiguousarray(
            Wo.reshape(NL, DT, 128, D).transpose(0, 2, 1, 3)
            .reshape(NL, 128, DT * D)).astype(bf16),
        "p_bo": np.ascontiguousarray(
            bo.reshape(NL, DT, 128).transpose(0, 2, 1)),
        "p_g1": np.ascontiguousarray(
            f32(inputs["ln1_g"]).reshape(NL, DT, 128).transpose(0, 2, 1)),
        "p_c1": np.ascontiguousarray(
            f32(inputs["ln1_b"]).reshape(NL, DT, 128).transpose(0, 2, 1)),
        "p_g2": np.ascontiguousarray(
            f32(inputs["ln2_g"]).reshape(NL, DT, 128).transpose(0, 2, 1)),
        "p_c2": np.ascontiguousarray(
            f32(inputs["ln2_b"]).reshape(NL, DT, 128).transpose(0, 2, 1)),
        "p_b1": np.ascontiguousarray(
            b1.reshape(NL, FC, 128).transpose(0, 2, 1)),
        "p_b2": np.ascontiguousarray(
            b2.reshape(NL, DT, 128).transpose(0, 2, 1)),
        "p_w1": np.ascontiguousarray(
            W1.reshape(NL, DT, 128, FC, 128).transpose(0, 3, 2, 1, 4)
            .reshape(NL, FC, 128, DT * 128)).astype(bf16),
        "p_w2": np.ascontiguousarray(
            W2.reshape(NL, FC, 128, DT, 128).transpose(0, 3, 2, 1, 4)
            .reshape(NL, DT, 128, FC * 128)).astype(bf16),
        "p_aw": np.ascontiguousarray(
            f32(inputs["action_W"]).reshape(DT, 128, A).transpose(1, 0, 2)
            .reshape(128, DT * A)).astype(bf16),
        "p_ab": np.ascontiguousarray(f32(inputs["action_b"]).reshape(A, 1)),
    }

    in_maps = []
    for c in range(NC):
        b, half = divmod(c, 2)
        g = (np.arange(S) + half * T) % S          # local col -> global token
        st_loc = np.zeros((SS, S), np.float32)
        m = (g >= 512) & (g < 1024)
        st_loc[:, m] = state[b, g[m] - 512, :].T
        ras = np.zeros((3, S), np.float32)
        m0 = g < 512
        ras[0, m0] = reward[b, g[m0], 0]
        m2 = g >= 1024
        ras[1, m2] = action[b, g[m2] - 1024, 0]
        ras[2, :] = np.sign(timestep[b, g % 512, 0] / float(D))
        maskv = np.ones((128, KT), np.float32)
        if half == 1:
            maskv[:, 2:6] = 0.0
        otherp = 1 - half
        offs = np.array([[otherp * D * T + dt * 128 * T for dt in range(DT)]],
                        dtype=np.uint32)
        m = dict(shared)
        m["p_state"] = st_loc.astype(bf16)
        m["p_ras"] = ras.astype(bf16)
        m["p_maskv"] = maskv
        m["p_offs"] = offs
        in_maps.append(m)
    return in_maps


def kernel(**inputs):
    nc, _ = _build()
    in_maps = _make_in_maps(inputs)
    res = run_bass_kernel_spmd(nc, in_maps, list(range(NC)))
    out = np.stack([res.results[2 * b + 1]["head"].T for b in range(B)])
    return out.astype(np.float32)
